# revision 7
# baseline (speedup 1.0000x reference)
"""Trainium2 Bass kernel for the differentiable-UKF forward pass.

Layout strategy (per core, pure data parallel over batch):
  * batch-major "AoS" SBUF masters [128, F, slots] for all per-element
    linear algebra (Cholesky, covariances, Kalman update), with batch
    b = p*F + f  (p = partition, f = free column).
  * PE-transposed "feature-major" tiles [128, F*16] for the sigma-point
    RK4/MLP pipeline: 8 chunks of 16 rows (6 state + 2 control + 8 pad),
    tokens along the free dim.  mm1 uses 4 block-structured K=128 weight
    variants (chunk-pair per variant), mm2 writes [32, 512] pieces at
    the 4 PSUM column quadrants.
  * RK4 is reformulated so only the combination
      A = -sigma + z2 + 2*z3 + z4 + (dt/2)*khat4  (= 3*(sigma_pred - dt*b2p))
    is carried; bias corrections for the missing b2 are folded into the
    per-stage tanh biases, and all fixed scale factors are folded into
    downstream scalar_tensor_tensor immediates.
"""
import numpy as np
from contextlib import ExitStack

import concourse.bass as bass
import concourse.bacc as bacc
import concourse.tile as tile
from concourse import mybir
from concourse.bass_utils import run_bass_kernel_spmd

F32 = mybir.dt.float32
AO = mybir.AluOpType
AF = mybir.ActivationFunctionType

N_CORES = 8
B_TOTAL = 131072
BC = B_TOTAL // N_CORES        # 16384 per core
NSUB = 2                       # sub-batches per core (pipeline stages)
BSUB = BC // NSUB              # 8192
FS = BSUB // 128               # 64 free cols per partition
NBLK = (FS * 16) // 128        # transpose blocks per feature tile (8)
NS = 13                        # sigma points

DT = 0.01
ALPHA, BETA, KAPPA = 0.5, 2.0, 0.0

# packed runtime-constant vector layout (cst tensor, [1, 104])
CST_Q = 0        # 36: Q
CST_Q4 = 36      # 36: Q + 1e-4 I
CST_R = 72       # 9:  R
CST_RST = 81     # 9:  R + 1e-5 I
CST_DTB2 = 90    # 6:  dt * b2
CST_LEN = 104

_PROG_CACHE = {}


def _host_consts(W1, b1, W2, b2, LQ, LR):
    f32 = np.float32
    W1 = np.asarray(W1, f32); b1 = np.asarray(b1, f32)
    W2 = np.asarray(W2, f32); b2 = np.asarray(b2, f32)
    LQ = np.asarray(LQ, f32); LR = np.asarray(LR, f32)

    W1e = np.zeros((16, 64), f32); W1e[:8] = W1
    # mm1 weight variants: variant q covers chunks (2q, 2q+1); K=128, M=128
    W1Q = np.zeros((4, 128, 128), f32)
    for q in range(4):
        W1Q[q, 16 * (2 * q):16 * (2 * q) + 16, 0:64] = W1e
        W1Q[q, 16 * (2 * q + 1):16 * (2 * q + 1) + 16, 64:128] = W1e
    W2p = np.zeros((64, 16), f32); W2p[:, :6] = W2
    W2bd = np.zeros((128, 32), f32)
    W2bd[0:64, 0:16] = W2p; W2bd[64:128, 16:32] = W2p

    bcorr = (b2 @ W1[:6]).astype(f32)          # b2 routed through state rows of W1
    bias1 = b1
    bias2 = (b1 + (DT / 2) * bcorr).astype(f32)
    bias4 = (b1 + DT * bcorr).astype(f32)
    biasT = np.zeros((128, 4), f32)
    for st, bb in enumerate((bias1, bias2, bias2, bias4)):
        biasT[0:64, st] = bb; biasT[64:128, st] = bb

    LQt = np.tril(LQ); Q = (LQt @ LQt.T + 1e-7 * np.eye(6)).astype(f32)
    LRt = np.tril(LR); R = (LRt @ LRt.T + 1e-7 * np.eye(3)).astype(f32)
    cst = np.zeros((1, CST_LEN), f32)
    cst[0, CST_Q:CST_Q + 36] = Q.reshape(-1)
    cst[0, CST_Q4:CST_Q4 + 36] = (Q + 1e-4 * np.eye(6, dtype=f32)).reshape(-1)
    cst[0, CST_R:CST_R + 9] = R.reshape(-1)
    cst[0, CST_RST:CST_RST + 9] = (R + 1e-5 * np.eye(3, dtype=f32)).reshape(-1)
    cst[0, CST_DTB2:CST_DTB2 + 6] = (DT * b2).astype(f32)
    eye = np.eye(128, dtype=f32)
    return dict(W1Q=W1Q.reshape(512, 128), W2bd=W2bd, biasT=biasT, cst=cst, eye=eye)


def _bcast(tile_ap, f_count, n):
    """[128, n] tile -> broadcast AP [128, f_count, n] with step-0 f dim."""
    return bass.AP(tensor=tile_ap.tensor, offset=tile_ap.offset,
                   ap=[tile_ap.ap[0], [0, f_count], [1, n]])


def _build_program():
    nc = bacc.Bacc("TRN2", target_bir_lowering=False, debug=False,
                   num_devices=N_CORES)

    def din(name, shape):
        return nc.dram_tensor(name, list(shape), F32, kind="ExternalInput").ap()

    def dout(name, shape):
        return nc.dram_tensor(name, list(shape), F32, kind="ExternalOutput").ap()

    x_d = din("x", (BC, 6)); P_d = din("P", (BC, 36))
    u_d = din("u", (BC, 2)); y_d = din("y", (BC, 3))
    W1Q_d = din("W1Q", (512, 128)); W2bd_d = din("W2bd", (128, 32))
    biasT_d = din("biasT", (128, 4)); cst_d = din("cst", (1, CST_LEN))
    eye_d = din("eye", (128, 128))

    xu_d = dout("xu", (BC, 6)); Pu_d = dout("Pu", (BC, 36))
    xp_d = dout("xp", (BC, 6)); Pp_d = dout("Pp", (BC, 36))
    yp_d = dout("yp", (BC, 3)); So_d = dout("So", (BC, 9))
    Ko_d = dout("Ko", (BC, 18))

    sfac = float(np.sqrt(np.float32(0.75)))
    eps_diag = 1e-5 / 0.75

    with tile.TileContext(nc) as tc:
        with ExitStack() as ctx:
            def pool(name, bufs, space="SBUF"):
                return ctx.enter_context(
                    tc.tile_pool(name=name, bufs=bufs, space=space))

            const = pool("const", 1)
            p_zA = pool("zA", 2)
            p_yA = pool("yA", 2)
            p_PA = pool("PA", 1)
            p_L = pool("L", 2)
            p_chol = pool("chol", 2)       # small chol temps [128, FS]
            p_sA = pool("sA", 2)
            p_sT = pool("sT", 2)
            p_zT = pool("zT", 2)           # z2T/z3T/z4T/A1 tags, Af reuses z3T
            p_hT = pool("hT", 2)
            p_AM = pool("AM", 2)
            p_cm = pool("cm", 1)           # C-phase medium tiles [128, FS, 6]
            p_prod = pool("prod", 2)
            p_gM = pool("gM", 1)
            p_out = pool("outm", 1)
            p_small = pool("small", 4)     # [128, FS] temps in C phase
            p_p3 = pool("p3", 4)           # [128, FS, 3] temps
            p_hps = pool("hps", 2, space="PSUM")   # [128, 1536] h regions
            p_psB = pool("psB", 2, space="PSUM")   # [128, 512] transpose/khat

            # ---- constants ----
            eye_t = const.tile([128, 128], F32, name="eye_t")
            nc.sync.dma_start(out=eye_t, in_=eye_d)
            W1Q_t = const.tile([128, 4, 128], F32, name="W1Q_t")
            nc.sync.dma_start(out=W1Q_t, in_=W1Q_d.rearrange("(a p) m -> p a m", p=128))
            W2bd_t = const.tile([128, 32], F32, name="W2bd_t")
            nc.sync.dma_start(out=W2bd_t, in_=W2bd_d)
            biasT_t = const.tile([128, 4], F32, name="biasT_t")
            nc.sync.dma_start(out=biasT_t, in_=biasT_d)
            cst_t = const.tile([128, CST_LEN], F32, name="cst_t")
            nc.sync.dma_start(out=cst_t, in_=bass.AP(
                tensor=cst_d.tensor, offset=cst_d.offset,
                ap=[[0, 128], [1, CST_LEN]]))

            subs = {}

            def phase_load_chol(sb):
                b0 = sb * BSUB
                st = {}
                zA = p_zA.tile([128, FS, 16], F32, name="zA")
                nc.vector.memset(zA.rearrange("p f q -> p (f q)"), 0.0)
                nc.sync.dma_start(
                    out=zA[:, :, 0:6],
                    in_=x_d[b0:b0 + BSUB, :].rearrange("(p f) q -> p f q", p=128))
                nc.sync.dma_start(
                    out=zA[:, :, 6:8],
                    in_=u_d[b0:b0 + BSUB, :].rearrange("(p f) q -> p f q", p=128))
                yA = p_yA.tile([128, FS, 3], F32, name="yA")
                nc.sync.dma_start(
                    out=yA,
                    in_=y_d[b0:b0 + BSUB, :].rearrange("(p f) q -> p f q", p=128))
                PA = p_PA.tile([128, FS, 36], F32, name="PA")
                nc.sync.dma_start(
                    out=PA,
                    in_=P_d[b0:b0 + BSUB, :].rearrange("(p f) q -> p f q", p=128))

                # L master, col-major: L[p, f, j, i] with i in 0..15 (rows >5 zero)
                L = p_L.tile([128, FS, 6, 8], F32, name="L")
                nc.vector.memset(L.rearrange("p f j q -> p (f j q)"), 0.0)

                # B matrix written into L slots (lower triangle), gpsimd
                for jj in range(6):
                    nc.vector.tensor_scalar(
                        out=L[:, :, jj, jj], in0=PA[:, :, 7 * jj],
                        scalar1=2.0, scalar2=eps_diag, op0=AO.mult, op1=AO.add)
                    for ii in range(jj + 1, 6):
                        nc.gpsimd.tensor_tensor(
                            out=L[:, :, jj, ii], in0=PA[:, :, 6 * ii + jj],
                            in1=PA[:, :, 6 * jj + ii], op=AO.add)

                # unrolled Cholesky on the L slots
                rrec = p_chol.tile([128, FS, 6], F32, tag="rrec", name="rrec")
                for jj in range(6):
                    # diagonal: d = B_jj - sum_k L_jk^2 (in place in L[.., jj, jj])
                    for kk in range(jj):
                        sq = p_chol.tile([128, FS], F32, tag="sq", name="sq")
                        nc.vector.tensor_mul(out=sq, in0=L[:, :, kk, jj],
                                             in1=L[:, :, kk, jj])
                        nc.vector.tensor_sub(out=L[:, :, jj, jj],
                                             in0=L[:, :, jj, jj], in1=sq)
                    nc.scalar.sqrt(out=L[:, :, jj, jj], in_=L[:, :, jj, jj])
                    nc.vector.reciprocal(out=rrec[:, :, jj], in_=L[:, :, jj, jj])
                    for ii in range(jj + 1, 6):
                        for kk in range(jj):
                            sq = p_chol.tile([128, FS], F32, tag="sq", name="sq")
                            nc.vector.tensor_mul(out=sq, in0=L[:, :, kk, ii],
                                                 in1=L[:, :, kk, jj])
                            nc.vector.tensor_sub(out=L[:, :, jj, ii],
                                                 in0=L[:, :, jj, ii], in1=sq)
                        nc.vector.tensor_mul(out=L[:, :, jj, ii],
                                             in0=L[:, :, jj, ii], in1=rrec[:, :, jj])
                st.update(zA=zA, yA=yA, L=L)
                subs[sb] = st
                return st

            def transpose_tile(flat_src, dst_consumer):
                """PE-transpose [128, FS*16] -> feed dst_consumer(psum_tile, t)
                for each of the 2 half tiles [128, 512] (4 blocks each)."""
                for t in range(NBLK // 4):
                    ps = p_psB.tile([128, 512], F32, tag="ps", name="psT")
                    for b in range(4):
                        blk = 4 * t + b
                        nc.tensor.transpose(
                            out=ps[:, 128 * b:128 * b + 128],
                            in_=flat_src[:, 128 * blk:128 * blk + 128],
                            identity=eye_t)
                    dst_consumer(ps, t)

            def phase_rk4(sb):
                st = subs[sb]
                zA, L = st["zA"], st["L"]
                AM = p_AM.tile([128, FS, 6, NS], F32, name="AM")
                zA_flat = zA.rearrange("p f q -> p (f q)")

                for s in range(NS):
                    # sigma point AoS tile
                    if s == 0:
                        sA_flat = zA_flat
                    else:
                        jj = (s - 1) % 6
                        sgn = sfac if s <= 6 else -sfac
                        sA = p_sA.tile([128, FS, 16], F32, name="sA")
                        sA_flat = sA.rearrange("p f q -> p (f q)")
                        nc.vector.memset(sA[:, :, 8:16], 0.0)
                        nc.vector.scalar_tensor_tensor(
                            out=sA[:, :, 0:8],
                            in0=L[:, :, jj, :],
                            scalar=sgn, in1=zA[:, :, 0:8], op0=AO.mult, op1=AO.add)

                    sT = p_sT.tile([128, FS * 16], F32, name="sT")

                    def to_sT(ps, t):
                        nc.vector.tensor_copy(out=sT[:, 512 * t:512 * t + 512], in_=ps)
                    transpose_tile(sA_flat, to_sT)

                    zcur = sT
                    ztiles = {}
                    for stage in range(1, 5):
                        csc = DT / 2 if stage < 3 else DT  # z_{st+1} = sig + c*khat
                        kps_halves = []
                        # 8 strips: (h,q) pairs grouped 3/3/2 into h-psum tiles
                        strips = [(h, q) for h in range(2) for q in range(4)]
                        hts = []
                        for grp in (strips[0:3], strips[3:6], strips[6:8]):
                            hp = p_hps.tile([128, 1536], F32, name="hp")
                            for k, (h, q) in enumerate(grp):
                                nc.tensor.matmul(
                                    out=hp[:, 512 * k:512 * k + 512],
                                    lhsT=W1Q_t[:, q, :],
                                    rhs=zcur[:, 512 * h:512 * h + 512],
                                    start=True, stop=True)
                            w = 512 * len(grp)
                            ht = p_hT.tile([128, 1536], F32, name="ht")
                            nc.scalar.activation(
                                out=ht[:, 0:w], in_=hp[:, 0:w], func=AF.Tanh,
                                bias=biasT_t[:, stage - 1:stage], scale=1.0)
                            hts.append((ht, grp))
                        # mm2: khat pieces per half
                        kps = {0: p_psB.tile([128, 512], F32, tag="ps", name="kps0"),
                               1: p_psB.tile([128, 512], F32, tag="ps", name="kps1")}
                        for ht, grp in hts:
                            for k, (h, q) in enumerate(grp):
                                nc.tensor.matmul(
                                    out=kps[h][32 * q:32 * q + 32, :],
                                    lhsT=W2bd_t,
                                    rhs=ht[:, 512 * k:512 * k + 512],
                                    start=True, stop=True,
                                    tile_position=(0, 32 * q))
                        if stage < 4:
                            zn = p_zT.tile([128, FS * 16], F32, tag=f"z{stage + 1}T", name=f"z{stage + 1}T")
                            for h in range(2):
                                nc.vector.scalar_tensor_tensor(
                                    out=zn[:, 512 * h:512 * h + 512],
                                    in0=kps[h], scalar=csc,
                                    in1=sT[:, 512 * h:512 * h + 512],
                                    op0=AO.mult, op1=AO.add)
                            ztiles[stage + 1] = zn
                            zcur = zn
                        else:
                            # A = -sig + z2 + 2 z3 + z4 + (dt/2) khat4
                            A1 = p_zT.tile([128, FS * 16], F32, tag="A1", name="A1")
                            nc.vector.scalar_tensor_tensor(
                                out=A1, in0=ztiles[3], scalar=2.0,
                                in1=ztiles[2], op0=AO.mult, op1=AO.add)
                            nc.vector.scalar_tensor_tensor(
                                out=A1, in0=sT, scalar=-1.0, in1=A1,
                                op0=AO.mult, op1=AO.add)
                            nc.gpsimd.tensor_tensor(
                                out=A1, in0=ztiles[4], in1=A1, op=AO.add)
                            Af = p_zT.tile([128, FS * 16], F32, tag="z3T", name="Af")
                            for h in range(2):
                                nc.vector.scalar_tensor_tensor(
                                    out=Af[:, 512 * h:512 * h + 512],
                                    in0=kps[h], scalar=DT / 2,
                                    in1=A1[:, 512 * h:512 * h + 512],
                                    op0=AO.mult, op1=AO.add)

                    # transpose Af back into AM[:, :, 0:6, s]
                    def to_AM(ps, t):
                        src = ps.rearrange("p (bf q) -> p bf q", q=16)[:, :, 0:6]
                        nc.vector.tensor_copy(
                            out=AM[:, 32 * t:32 * t + 32, :, s], in_=src)
                    transpose_tile(Af, to_AM)
                st["AM"] = AM

            def phase_cov(sb):
                st = subs[sb]
                AM, yA = st["AM"], st["yA"]
                tSA = p_cm.tile([128, FS, 6], F32, tag="tSA", name="tSA")
                nc.vector.tensor_reduce(out=tSA, in_=AM,
                                        axis=mybir.AxisListType.X, op=AO.add)
                tT = p_cm.tile([128, FS, 6], F32, tag="tT", name="tT")
                nc.vector.scalar_tensor_tensor(
                    out=tT, in0=AM[:, :, :, 0], scalar=-10.0, in1=tSA,
                    op0=AO.mult, op1=AO.add)
                for s in range(NS):
                    nc.vector.scalar_tensor_tensor(
                        out=AM[:, :, :, s], in0=tT, scalar=-1.0 / 3.0,
                        in1=AM[:, :, :, s], op0=AO.mult, op1=AO.add)
                xpM = p_cm.tile([128, FS, 6], F32, tag="xpM", name="xpM")
                nc.vector.scalar_tensor_tensor(
                    out=xpM, in0=tT, scalar=1.0 / 9.0,
                    in1=_bcast(cst_t[:, CST_DTB2:CST_DTB2 + 6], FS, 6),
                    op0=AO.mult, op1=AO.add)

                gM = p_gM.tile([128, FS, 36], F32, name="gM")
                for i in range(6):
                    for j in range(i, 6):
                        prod = p_prod.tile([128, FS, NS], F32, name="prod")
                        nc.gpsimd.tensor_tensor(
                            out=prod, in0=AM[:, :, i, :], in1=AM[:, :, j, :],
                            op=AO.mult)
                        nc.vector.tensor_reduce(
                            out=gM[:, :, 6 * i + j], in_=prod,
                            axis=mybir.AxisListType.X, op=AO.add)
                        nc.vector.scalar_tensor_tensor(
                            out=gM[:, :, 6 * i + j], in0=prod[:, :, 0],
                            scalar=-7.0 / 4.0, in1=gM[:, :, 6 * i + j],
                            op0=AO.mult, op1=AO.add)
                        if i != j:
                            nc.gpsimd.tensor_copy(out=gM[:, :, 6 * j + i],
                                                  in_=gM[:, :, 6 * i + j])

                b0 = sb * BSUB
                PpM = p_out.tile([128, FS, 36], F32, tag="PpM", name="PpM")
                nc.vector.scalar_tensor_tensor(
                    out=PpM.rearrange("p f q -> p (f q)"),
                    in0=gM.rearrange("p f q -> p (f q)"), scalar=1.0 / 27.0,
                    in1=_bcast(cst_t[:, CST_Q:CST_Q + 36], FS, 36),
                    op0=AO.mult, op1=AO.add)
                nc.sync.dma_start(
                    out=Pp_d[b0:b0 + BSUB, :].rearrange("(p f) q -> p f q", p=128),
                    in_=PpM)

                gM4 = gM.rearrange("p f (i j) -> p f i j", i=6)
                sM = p_cm.tile([128, FS, 9], F32, tag="sM", name="sM")
                SoM = p_out.tile([128, FS, 9], F32, tag="SoM", name="SoM")
                for i in range(3):
                    nc.vector.scalar_tensor_tensor(
                        out=sM[:, :, 3 * i:3 * i + 3],
                        in0=gM[:, :, 6 * i:6 * i + 3], scalar=1.0 / 27.0,
                        in1=_bcast(cst_t[:, CST_RST + 3 * i:CST_RST + 3 * i + 3], FS, 3),
                        op0=AO.mult, op1=AO.add)
                    nc.vector.scalar_tensor_tensor(
                        out=SoM[:, :, 3 * i:3 * i + 3],
                        in0=gM[:, :, 6 * i:6 * i + 3], scalar=1.0 / 27.0,
                        in1=_bcast(cst_t[:, CST_R + 3 * i:CST_R + 3 * i + 3], FS, 3),
                        op0=AO.mult, op1=AO.add)
                nc.sync.dma_start(
                    out=So_d[b0:b0 + BSUB, :].rearrange("(p f) q -> p f q", p=128),
                    in_=SoM)

                # 3x3 symmetric inverse (adjugate), scaled by 1/27
                sv = sM  # slots: 0..8 row-major
                cof = p_cm.tile([128, FS, 6], F32, tag="cof", name="cof")
                tmpa = p_small.tile([128, FS], F32, tag="tmpa", name="tmpa")
                tmpb = p_small.tile([128, FS], F32, tag="tmpb", name="tmpb")

                def s_(i, j):
                    return sv[:, :, 3 * i + j]

                cdefs = [  # (slot, (a,b),(c,d),(e,f),(g,h)): cof = s_ab*s_cd - s_ef*s_gh
                    (0, (1, 1), (2, 2), (1, 2), (1, 2)),   # cof00
                    (1, (0, 2), (1, 2), (0, 1), (2, 2)),   # cof01
                    (2, (0, 1), (1, 2), (0, 2), (1, 1)),   # cof02
                    (3, (0, 0), (2, 2), (0, 2), (0, 2)),   # cof11
                    (4, (0, 1), (0, 2), (0, 0), (1, 2)),   # cof12
                    (5, (0, 0), (1, 1), (0, 1), (0, 1)),   # cof22
                ]
                for slot, ab, cd, ef, gh in cdefs:
                    nc.gpsimd.tensor_tensor(out=tmpa, in0=s_(*ab), in1=s_(*cd),
                                            op=AO.mult)
                    nc.gpsimd.tensor_tensor(out=tmpb, in0=s_(*ef), in1=s_(*gh),
                                            op=AO.mult)
                    nc.vector.tensor_sub(out=cof[:, :, slot], in0=tmpa, in1=tmpb)
                det = p_small.tile([128, FS], F32, tag="det", name="det")
                nc.vector.tensor_mul(out=det, in0=s_(0, 0), in1=cof[:, :, 0])
                for sl, (i, j) in ((1, (0, 1)), (2, (0, 2))):
                    nc.gpsimd.tensor_tensor(out=tmpa, in0=s_(i, j),
                                            in1=cof[:, :, sl], op=AO.mult)
                    nc.vector.tensor_add(out=det, in0=det, in1=tmpa)
                rdet = p_small.tile([128, FS], F32, tag="rdet", name="rdet")
                nc.vector.reciprocal(out=rdet, in_=det)
                nc.vector.tensor_scalar_mul(out=rdet, in0=rdet, scalar1=1.0 / 27.0)
                Sinv = p_cm.tile([128, FS, 9], F32, tag="Sinv", name="Sinv")
                for slot, (i, j) in enumerate(((0, 0), (0, 1), (0, 2),
                                               (1, 1), (1, 2), (2, 2))):
                    nc.vector.tensor_mul(out=Sinv[:, :, 3 * i + j],
                                         in0=cof[:, :, slot], in1=rdet)
                    if i != j:
                        nc.gpsimd.tensor_copy(out=Sinv[:, :, 3 * j + i],
                                              in_=Sinv[:, :, 3 * i + j])
                Sinv3 = Sinv.rearrange("p f (i j) -> p f i j", i=3)

                # K = (G/27) @ inv(Sst) = G @ Sinv27
                kM = p_out.tile([128, FS, 6, 3], F32, tag="kM", name="kM")
                for i in range(6):
                    for j in range(3):
                        p3 = p_p3.tile([128, FS, 3], F32, name="p3")
                        nc.gpsimd.tensor_tensor(
                            out=p3, in0=gM4[:, :, i, 0:3], in1=Sinv3[:, :, :, j],
                            op=AO.mult)
                        nc.vector.tensor_reduce(
                            out=kM[:, :, i, j], in_=p3,
                            axis=mybir.AxisListType.X, op=AO.add)
                nc.sync.dma_start(
                    out=Ko_d[b0:b0 + BSUB, :].rearrange("(p f) q -> p f q", p=128),
                    in_=kM.rearrange("p f i j -> p f (i j)"))

                # innovation, x_upd
                iM = p_cm.tile([128, FS, 3], F32, tag="iM", name="iM")
                nc.vector.scalar_tensor_tensor(
                    out=iM, in0=xpM[:, :, 0:3], scalar=-1.0, in1=yA,
                    op0=AO.mult, op1=AO.add)
                xuM = p_out.tile([128, FS, 6], F32, tag="xuM", name="xuM")
                for i in range(6):
                    p3 = p_p3.tile([128, FS, 3], F32, name="p3")
                    nc.gpsimd.tensor_tensor(out=p3, in0=kM[:, :, i, :], in1=iM,
                                            op=AO.mult)
                    tmp1 = p_small.tile([128, FS], F32, tag="tmp1", name="tmp1")
                    nc.vector.tensor_reduce(out=tmp1, in_=p3,
                                            axis=mybir.AxisListType.X, op=AO.add)
                    nc.vector.tensor_add(out=xuM[:, :, i], in0=tmp1,
                                         in1=xpM[:, :, i])
                nc.sync.dma_start(
                    out=xu_d[b0:b0 + BSUB, :].rearrange("(p f) q -> p f q", p=128),
                    in_=xuM)
                nc.sync.dma_start(
                    out=xp_d[b0:b0 + BSUB, :].rearrange("(p f) q -> p f q", p=128),
                    in_=xpM)
                nc.sync.dma_start(
                    out=yp_d[b0:b0 + BSUB, :].rearrange("(p f) q -> p f q", p=128),
                    in_=xpM[:, :, 0:3])

                # KS = K @ S_true, P_upd = G/27 - KS K^T + (Q + 1e-4 I)
                So3 = SoM.rearrange("p f (i j) -> p f i j", i=3)
                ksM = p_out.tile([128, FS, 6, 3], F32, tag="ksM", name="ksM")
                for i in range(6):
                    for j in range(3):
                        p3 = p_p3.tile([128, FS, 3], F32, name="p3")
                        nc.gpsimd.tensor_tensor(
                            out=p3, in0=kM[:, :, i, :], in1=So3[:, :, :, j],
                            op=AO.mult)
                        nc.vector.tensor_reduce(
                            out=ksM[:, :, i, j], in_=p3,
                            axis=mybir.AxisListType.X, op=AO.add)
                PuM = p_out.tile([128, FS, 36], F32, tag="PpM", name="PuM")
                for i in range(6):
                    for j in range(i, 6):
                        p3 = p_p3.tile([128, FS, 3], F32, name="p3")
                        nc.gpsimd.tensor_tensor(
                            out=p3, in0=ksM[:, :, i, :], in1=kM[:, :, j, :],
                            op=AO.mult)
                        tmp1 = p_small.tile([128, FS], F32, tag="tmp1", name="tmp1")
                        nc.vector.tensor_reduce(out=tmp1, in_=p3,
                                                axis=mybir.AxisListType.X, op=AO.add)
                        nc.vector.scalar_tensor_tensor(
                            out=PuM[:, :, 6 * i + j], in0=gM[:, :, 6 * i + j],
                            scalar=1.0 / 27.0, in1=tmp1,
                            op0=AO.mult, op1=AO.subtract)
                        if i != j:
                            nc.gpsimd.tensor_copy(out=PuM[:, :, 6 * j + i],
                                                  in_=PuM[:, :, 6 * i + j])
                nc.vector.tensor_add(
                    out=PuM.rearrange("p f q -> p (f q)"),
                    in0=PuM.rearrange("p f q -> p (f q)"),
                    in1=_bcast(cst_t[:, CST_Q4:CST_Q4 + 36], FS, 36))
                nc.sync.dma_start(
                    out=Pu_d[b0:b0 + BSUB, :].rearrange("(p f) q -> p f q", p=128),
                    in_=PuM)

            # emission order for cross-sub pipelining:
            # A0 B0 | A1 C0-interleaved-with-B1 | C1
            phase_load_chol(0)
            phase_rk4(0)
            phase_load_chol(1)
            phase_rk4(1)
            phase_cov(0)
            phase_cov(1)

    nc.compile()
    return nc


def _get_program():
    if "nc" not in _PROG_CACHE:
        _PROG_CACHE["nc"] = _build_program()
    return _PROG_CACHE["nc"]


def kernel(x, P, u, y, LQ, LR, W1, b1, W2, b2):
    f32 = np.float32
    x = np.ascontiguousarray(x, f32); P = np.ascontiguousarray(P, f32)
    u = np.ascontiguousarray(u, f32); y = np.ascontiguousarray(y, f32)
    consts = _host_consts(W1, b1, W2, b2, LQ, LR)
    nc = _get_program()

    in_maps = []
    for c in range(N_CORES):
        b0, b1_ = c * BC, (c + 1) * BC
        in_maps.append(dict(
            x=x[b0:b1_], P=P[b0:b1_].reshape(BC, 36), u=u[b0:b1_], y=y[b0:b1_],
            W1Q=consts["W1Q"], W2bd=consts["W2bd"], biasT=consts["biasT"],
            cst=consts["cst"], eye=consts["eye"]))
    res = run_bass_kernel_spmd(nc, in_maps, core_ids=list(range(N_CORES)))

    def cat(name):
        return np.concatenate([res.results[c][name] for c in range(N_CORES)], axis=0)

    x_upd = cat("xu")
    P_upd = cat("Pu").reshape(B_TOTAL, 6, 6)
    x_pred = cat("xp")
    P_pred = cat("Pp").reshape(B_TOTAL, 6, 6)
    y_pred = cat("yp")
    S = cat("So").reshape(B_TOTAL, 3, 3)
    K = cat("Ko").reshape(B_TOTAL, 6, 3)
    return x_upd, P_upd, x_pred, P_pred, y_pred, S, K
